# revision 2
# baseline (speedup 1.0000x reference)
"""2-layer GCN on 8 trn2 cores — v2: chunked AllGather overlapped with
bucket-major SpMM.

Design (row-sharded):
  - Core m owns node rows [m*RPC, (m+1)*RPC), padded to RPAD=12544.
  - GEMM1: Z1 = X@W1+b1 per-core, row-major bf16 in DRAM (z1_loc).
  - Z exchange: 4 chunked AllGathers over row-ranges of z_loc (CHR=3136 rows
    each).  Chunk j output z_c[j] = [8*CHR, 128] (shard-major) — edges are
    bucketed by source row-range j so bucket-j SpMM work only gates on AG j.
    Real-HW probe: full 25.7MB AG ~150us, 6.4MB chunk AG ~42us => chunked
    pipeline hides most of the exchange under gather/matmul work.
  - SpMM bucket-major: for j: for g (block groups): one dma_gather from
    z_c[j] (<=3072 descs, SWDGE ring = 49152B), per 128-slot chunk a DVE
    tensor_scalar builds S[slot,row]=(iota==rloc)*val and PE accumulates
    psum[feat, group_rows] += G^T @ S.  Per-bucket psum is folded into an
    SBUF f32 accumulator (DVE add), so psum lifetime = (group, bucket) and
    buckets can sweep all blocks phase by phase.
  - After the last bucket phase per group: relu+cast -> GEMM2 for those
    blocks -> z2_loc write, so AG2 chunks launch while SpMM1 still runs.
  - Layer 2 repeats SpMM with z2 chunks; acc (f32, feature-major) is DMA'd
    out directly.

Static-vs-dynamic: program layout (caps, groups, reg counts) is computed
from max-over-cores segment sizes so one SPMD program serves all 8 cores;
per-core variation lives in the idx/rloc/sval input tables.  Gather calls
use num_idxs_reg < num_idxs with trailing -1 idxs to skip tail pad slots.
"""

import sys

import numpy as np
import ml_dtypes

_TRN_REPO = "/opt/trn_rl_repo"
if _TRN_REPO not in sys.path:
    sys.path.insert(0, _TRN_REPO)

import concourse.bass as bass
import concourse.tile as tile
from concourse import bacc, mybir
from concourse.bass_utils import run_bass_kernel_spmd

BF16 = mybir.dt.bfloat16
F32 = mybir.dt.float32
I16 = mybir.dt.int16


class Cfg:
    def __init__(self, n_nodes, in_size, hidden, out_size):
        self.M = 8
        self.NN = n_nodes
        self.IN = in_size
        self.HID = hidden
        self.OUT = out_size
        assert n_nodes % self.M == 0
        self.RPC = n_nodes // self.M          # real rows per core (12500)
        self.BL = 128
        self.NB = (self.RPC + 127) // 128     # 98 blocks
        self.RPAD = self.NB * 128             # 12544
        self.J = 4                            # AG chunks / edge buckets
        assert self.RPAD % self.J == 0
        self.CHR = self.RPAD // self.J        # 3136 rows per chunk
        self.CHOUT = self.M * self.CHR        # 25088 (int16-safe)
        assert self.CHOUT <= 32768
        self.RINGB = 16384                    # runtime-pinned SWDGE ring
        self.RINGD = self.RINGB // 16         # 1024 descs per gather call
        self.MAXGB = 8                        # psum tile = [128, MAXGB*128] f32
        self.WCH = 32                         # max chunks per window (gb tile)
        self.KIN = in_size // 128
        self.GGB = 8                          # gemm block group
        assert in_size % 128 == 0 and hidden == 128 and out_size == 128


FULL = Cfg(100000, 256, 128, 128)


def build_plan(cfg, row, col, vals):
    row = np.asarray(row).astype(np.int64)
    col = np.asarray(col).astype(np.int64)
    vals = np.asarray(vals).astype(np.float32)

    m_e = row // cfg.RPC                      # dest core
    er = row % cfg.RPC
    blk = er // cfg.BL
    rloc_e = er % cfg.BL
    s_e = col // cfg.RPC                      # source core
    r0 = col % cfg.RPC
    j_e = r0 // cfg.CHR                       # bucket
    cidx_e = s_e * cfg.CHR + (r0 - j_e * cfg.CHR)   # idx into z_c[j]

    NB, J = cfg.NB, cfg.J
    # per-core per-(block, bucket) counts
    counts = np.zeros((cfg.M, NB * J), dtype=np.int64)
    key = blk * J + j_e
    for m in range(cfg.M):
        sel = m_e == m
        if sel.any():
            counts[m] = np.bincount(key[sel], minlength=NB * J)
    need = counts.max(axis=0).reshape(NB, J)
    cap = np.maximum(-(-need // cfg.BL), 1)   # chunks per (b, j); >=1

    # greedy windows per bucket: consecutive blocks, <= WCH chunks, <= MAXGB
    groups = []                               # groups[j] = list of block-lists
    for j in range(J):
        gj, cur, ch = [], [], 0
        for b in range(NB):
            c = int(cap[b, j])
            if cur and (ch + c > cfg.WCH or len(cur) >= cfg.MAXGB):
                gj.append(cur)
                cur, ch = [], 0
            cur.append(b)
            ch += c
        if cur:
            gj.append(cur)
        groups.append(gj)

    # slot layout per window; gather calls split linearly at RINGD descs
    slot_off = {}
    calls = {}                                # (j, gi) -> list of (o, n, reg)
    off = 0
    for j in range(J):
        for gi, blist in enumerate(groups[j]):
            ioff = off
            for b in blist:
                slot_off[(b, j)] = off
                off += int(cap[b, j]) * cfg.BL
            ndesc = off - ioff
            last = blist[-1]
            wreg = ndesc - int(cap[last, j]) * cfg.BL + int(need[last, j])
            cl = []
            o = 0
            while o < ndesc:
                n = min(cfg.RINGD, ndesc - o)
                r = max(0, min(n, wreg - o))
                if r > 0:
                    cl.append((o, n, r))
                o += n
            calls[(j, gi)] = (ioff, ndesc, cl)
    nslot = off
    nchunk = nslot // cfg.BL

    per_core = []
    for m in range(cfg.M):
        sel = m_e == m
        eb = blk[sel]
        ej = j_e[sel]
        ec = cidx_e[sel]
        ev = vals[sel]
        erl = rloc_e[sel]
        order = np.lexsort((ec, ej, eb))
        eb, ej, ec, ev, erl = (a[order] for a in (eb, ej, ec, ev, erl))

        idx16 = np.zeros(nslot, dtype=np.int16)
        rl = np.zeros(nslot, dtype=np.float32)
        sv = np.zeros(nslot, dtype=np.float32)

        k2 = eb * J + ej
        bstart = np.searchsorted(k2, np.arange(NB * J + 1))
        for b in range(NB):
            for j in range(J):
                i0, i1 = bstart[b * J + j], bstart[b * J + j + 1]
                n = i1 - i0
                so = slot_off[(b, j)]
                assert n <= cap[b, j] * cfg.BL
                idx16[so:so + n] = ec[i0:i1].astype(np.int16)
                rl[so:so + n] = erl[i0:i1].astype(np.float32)
                sv[so:so + n] = ev[i0:i1]
        # tail trim: mark slots beyond each call's reg as -1
        for (ioff, ndesc, cl) in calls.values():
            for (o, n, r) in cl:
                idx16[ioff + o + r: ioff + o + n] = -1
            covered = sum(n for (o, n, r) in cl)
            # calls fully in the pad tail are skipped: mark their slots too
            idx16[ioff + covered: ioff + ndesc] = -1
        idx_w = np.tile(idx16.reshape(-1, 16).T, (8, 1))
        rl_w = rl.reshape(nchunk, cfg.BL).T
        sv_w = sv.reshape(nchunk, cfg.BL).T
        per_core.append(dict(idx=np.ascontiguousarray(idx_w),
                             rloc=np.ascontiguousarray(rl_w),
                             sval=np.ascontiguousarray(sv_w)))
    return dict(groups=groups, cap=cap, slot_off=slot_off, calls=calls,
                nslot=nslot, nchunk=nchunk, per_core=per_core)


def build_program(cfg, plan):
    groups, cap, slot_off = plan["groups"], plan["cap"], plan["slot_off"]
    calls, nslot, nchunk = plan["calls"], plan["nslot"], plan["nchunk"]

    nc = bacc.Bacc("TRN2", target_bir_lowering=False, debug=False,
                   num_devices=cfg.M, dynamic_dma_scratch_size=cfg.RINGB)

    xt_d = nc.dram_tensor("xt", [cfg.IN, cfg.RPAD], BF16, kind="ExternalInput")
    wcols = cfg.KIN * 128 + 128 + 4 * 128
    wpack_d = nc.dram_tensor("wpack", [128, wcols], BF16, kind="ExternalInput")
    idx_d = nc.dram_tensor("idx", [128, nslot // 16], I16, kind="ExternalInput")
    fcols = 2 * nchunk
    fpack_d = nc.dram_tensor("fpack", [128, fcols], F32, kind="ExternalInput")
    out_d = nc.dram_tensor("out", [128, cfg.RPAD], F32, kind="ExternalOutput")

    z1_loc = nc.dram_tensor("z1_loc", [cfg.RPAD, cfg.HID], BF16)
    z2_loc = nc.dram_tensor("z2_loc", [cfg.RPAD, cfg.HID], BF16)
    # note: addr_space="Shared" AG outputs crash this NRT runtime
    # (NRT_EXEC_UNIT_UNRECOVERABLE); Local measured ~42us per 6.4MB chunk AG.
    z1c = [nc.dram_tensor(f"z1c{j}", [cfg.CHOUT, cfg.HID], BF16)
           for j in range(cfg.J)]
    z2c = [nc.dram_tensor(f"z2c{j}", [cfg.CHOUT, cfg.HID], BF16)
           for j in range(cfg.J)]

    rg = [list(range(cfg.M))]

    with tile.TileContext(nc) as tc:
        from contextlib import ExitStack
        with ExitStack() as ctx:
            const = ctx.enter_context(tc.tile_pool(name="const", bufs=1))
            acc_pool = ctx.enter_context(tc.tile_pool(name="acc", bufs=1))
            xt_pool = ctx.enter_context(tc.tile_pool(name="xt", bufs=4))
            zs_pool = ctx.enter_context(tc.tile_pool(name="zs", bufs=2))
            rtb_pool = ctx.enter_context(tc.tile_pool(name="rtb", bufs=2))
            s_pool = ctx.enter_context(tc.tile_pool(name="sm", bufs=4))
            psum_g = ctx.enter_context(
                tc.tile_pool(name="psum_g", bufs=2, space="PSUM"))
            psum_s = ctx.enter_context(
                tc.tile_pool(name="psum_s", bufs=2, space="PSUM"))

            # resident constants
            wpack_sb = const.tile([128, wcols], BF16, tag="wpack", name="wp")
            nc.sync.dma_start(wpack_sb[:], wpack_d[:, :])
            w1_sb = [wpack_sb[:, k * 128:(k + 1) * 128] for k in range(cfg.KIN)]
            o = cfg.KIN * 128
            w2_sb = wpack_sb[:, o:o + 128]
            b1_sb = wpack_sb[0:1, o + 128:o + 256]
            b2_sb = wpack_sb[0:1, o + 256:o + 384]
            ones_sb = wpack_sb[0:1, o + 384:o + 512]
            iota_sb = wpack_sb[:, o + 512:o + 640]
            idx_sb = const.tile([128, nslot // 16], I16, tag="idx", name="ix")
            nc.sync.dma_start(idx_sb[:], idx_d[:, :])
            fpack_sb = const.tile([128, fcols], F32, tag="fpack", name="fp")
            nc.sync.dma_start(fpack_sb[:], fpack_d[:, :])
            rloc_sb = fpack_sb[:, 0:nchunk]
            sval_sb = fpack_sb[:, nchunk:2 * nchunk]

            acc = acc_pool.tile([128, cfg.RPAD], F32, tag="acc", name="acc")

            # two explicit long-lived gather buffers (manual double-buffer):
            # memset once so tail slots skipped by num_idxs_reg stay finite
            gbufs = [const.tile([128, cfg.WCH, 128], BF16, tag=f"gbuf{i}",
                                name=f"gb{i}") for i in range(2)]
            for t in gbufs:
                nc.vector.memset(t.rearrange("p c f -> p (c f)")[:, :], 0.0)

            # ---- GEMM1 -> z1_loc (+ AG1 chunk launches) ----
            ggroups = [list(range(g, min(g + cfg.GGB, cfg.NB)))
                       for g in range(0, cfg.NB, cfg.GGB)]

            def gemm_group(blist, lhsT_cols, w_list, bias, zdst, relu_src=None):
                """lhsT_cols(b) -> list of [128,128] lhsT tiles per k."""
                nbl = len(blist)
                zs = zs_pool.tile([128, cfg.GGB * 128], BF16, tag="zs",
                                  name="zs")
                for bi, b in enumerate(blist):
                    ps = psum_g.tile([128, 128], F32, tag="gps", name="gps")
                    tiles = lhsT_cols(b)
                    for k, (lt, wk) in enumerate(zip(tiles, w_list)):
                        nc.tensor.matmul(ps[:], lt, wk, start=(k == 0),
                                         stop=False, skip_group_check=True)
                    nc.tensor.matmul(ps[:], ones_sb, bias, start=False,
                                     stop=True, skip_group_check=True)
                    nc.scalar.copy(zs[:, bi * 128:(bi + 1) * 128], ps[:])
                t0 = blist[0]
                nc.sync.dma_start(
                    zdst.rearrange("(t p) f -> p t f", p=128)[:, t0:t0 + nbl, :],
                    zs.rearrange("p (t f) -> p t f", f=128)[:, :nbl, :])

            def x_cols(g0, nbl):
                tiles = []
                for k in range(cfg.KIN):
                    xt = xt_pool.tile([128, cfg.GGB * 128], BF16, tag="xt",
                                      name="xt")
                    nc.sync.dma_start(
                        xt[:, :nbl * 128],
                        xt_d[k * 128:(k + 1) * 128,
                             g0 * 128:(g0 + nbl) * 128])
                    tiles.append(xt)
                return tiles

            ag_rows_done = 0
            ag_emitted = 0

            def maybe_ag(zloc, zcs, rows_done):
                nonlocal ag_emitted
                while (ag_emitted < cfg.J and
                       rows_done >= (ag_emitted + 1) * cfg.CHR):
                    j = ag_emitted
                    nc.gpsimd.collective_compute(
                        "AllGather", mybir.AluOpType.bypass, replica_groups=rg,
                        ins=[zloc[j * cfg.CHR:(j + 1) * cfg.CHR, :]],
                        outs=[zcs[j][:, :]])
                    ag_emitted += 1

            for blist in ggroups:
                g0, nbl = blist[0], len(blist)
                tiles = x_cols(g0, nbl)

                def lhsT_cols(b, tiles=tiles, g0=g0):
                    return [t[:, (b - g0) * 128:(b - g0 + 1) * 128]
                            for t in tiles]

                gemm_group(blist, lhsT_cols, w1_sb, b1_sb, z1_loc)
                ag_rows_done = (blist[-1] + 1) * 128
                maybe_ag(z1_loc, z1c, ag_rows_done)

            # ---- SpMM (bucket-major) ----
            def spmm(zcs, layer):
                nonlocal ag_emitted
                if layer == 1:
                    ag_emitted = 0
                    z2_rows_done = 0
                ci = 0
                for j in range(cfg.J):
                    for gi, blist in enumerate(groups[j]):
                        ioff, ndesc, cl = calls[(j, gi)]
                        gcols = len(blist) * 128
                        c0 = blist[0] * 128
                        gb3 = gbufs[ci % 2]
                        ci += 1
                        gb = gb3.rearrange("p c f -> p (c f)")
                        for (o, n, r) in cl:
                            c_lo = o // 128
                            c_hi = -(-(o + n) // 128)
                            nc.gpsimd.dma_gather(
                                out_ap=gb3[:, c_lo:c_hi, :],
                                in_ap=zcs[j][:, :],
                                idxs_ap=idx_sb[:, (ioff + o) // 16:
                                               -(-(ioff + o + n) // 16)],
                                num_idxs=n, num_idxs_reg=r,
                                elem_size=cfg.HID)
                        ps = psum_s.tile([128, cfg.MAXGB * 128], F32,
                                         tag="sps", name="sps")
                        for b in blist:
                            boff = (b - blist[0]) * 128
                            ncc = int(cap[b, j])
                            so0 = slot_off[(b, j)] - ioff
                            for c in range(ncc):
                                cg = (slot_off[(b, j)] + c * 128) // 128
                                s = s_pool.tile([128, 128], BF16, tag="s",
                                                name="s")
                                nc.vector.tensor_scalar(
                                    s[:], iota_sb,
                                    rloc_sb[:, cg:cg + 1],
                                    sval_sb[:, cg:cg + 1],
                                    mybir.AluOpType.is_equal,
                                    mybir.AluOpType.mult)
                                nc.tensor.matmul(
                                    ps[:, boff:boff + 128],
                                    gb[:, (so0 + c * 128):(so0 + c * 128) + 128],
                                    s[:], start=(c == 0), stop=(c == ncc - 1),
                                    skip_group_check=True)
                        if j == 0:
                            nc.scalar.copy(acc[:, c0:c0 + gcols],
                                           ps[:, :gcols])
                        else:
                            nc.vector.tensor_tensor(
                                acc[:, c0:c0 + gcols], acc[:, c0:c0 + gcols],
                                ps[:, :gcols], mybir.AluOpType.add)
                        if j == cfg.J - 1:
                            # group finalized
                            if layer == 1:
                                # relu+cast, GEMM2, z2 write, maybe AG2
                                rtb = rtb_pool.tile([128, cfg.MAXGB * 128],
                                                    BF16, tag="rtb", name="rt")
                                nc.scalar.activation(
                                    rtb[:, :gcols], acc[:, c0:c0 + gcols],
                                    mybir.ActivationFunctionType.Relu)

                                def lhsT_cols(b, rtb=rtb, blist=blist):
                                    return [rtb[:, (b - blist[0]) * 128:
                                                (b - blist[0] + 1) * 128]]

                                gemm_group2(blist, lhsT_cols)
                                z2_rows_done = (blist[-1] + 1) * 128
                                maybe_ag(z2_loc, z2c, z2_rows_done)
                            else:
                                nc.sync.dma_start(out_d[:, c0:c0 + gcols],
                                                  acc[:, c0:c0 + gcols])

            def gemm_group2(blist, lhsT_cols):
                nbl = len(blist)
                zs = zs_pool.tile([128, cfg.GGB * 128], BF16, tag="zs",
                                  name="zs")
                for bi, b in enumerate(blist):
                    ps = psum_g.tile([128, 128], F32, tag="gps", name="gps")
                    nc.tensor.matmul(ps[:], lhsT_cols(b)[0], w2_sb,
                                     start=True, stop=False,
                                     skip_group_check=True)
                    nc.tensor.matmul(ps[:], ones_sb, b2_sb, start=False,
                                     stop=True, skip_group_check=True)
                    nc.scalar.copy(zs[:, bi * 128:(bi + 1) * 128], ps[:])
                t0 = blist[0]
                nc.sync.dma_start(
                    z2_loc.rearrange("(t p) f -> p t f", p=128)[:, t0:t0 + nbl, :],
                    zs.rearrange("p (t f) -> p t f", f=128)[:, :nbl, :])

            spmm(z1c, 1)
            spmm(z2c, 2)

    nc.compile()
    return nc


def prep_inputs(cfg, X, W1, b1, W2, b2, plan):
    bf = ml_dtypes.bfloat16
    nchunk = plan["nchunk"]
    per_core = plan["per_core"]
    wcols = cfg.KIN * 128 + 128 + 4 * 128
    wpack = np.zeros((128, wcols), dtype=np.float32)
    for k in range(cfg.KIN):
        wpack[:, k * 128:(k + 1) * 128] = np.asarray(W1)[k * 128:(k + 1) * 128]
    o = cfg.KIN * 128
    wpack[:, o:o + 128] = np.asarray(W2)
    wpack[0, o + 128:o + 256] = np.asarray(b1)
    wpack[0, o + 256:o + 384] = np.asarray(b2)
    wpack[0, o + 384:o + 512] = 1.0
    wpack[:, o + 512:o + 640] = np.arange(128, dtype=np.float32)[None, :]
    wpack = wpack.astype(bf)

    X = np.asarray(X).astype(np.float32)
    in_maps = []
    for m in range(cfg.M):
        xs = np.zeros((cfg.IN, cfg.RPAD), dtype=np.float32)
        xs[:, :cfg.RPC] = X[m * cfg.RPC:(m + 1) * cfg.RPC].T
        fpack = np.zeros((128, 2 * nchunk), dtype=np.float32)
        fpack[:, :nchunk] = per_core[m]["rloc"]
        fpack[:, nchunk:] = per_core[m]["sval"]
        in_maps.append(dict(
            xt=np.ascontiguousarray(xs.astype(bf)), wpack=wpack,
            idx=per_core[m]["idx"], fpack=fpack))
    return in_maps


def make(cfg, d):
    plan = build_plan(cfg, d["row"], d["col"], d["vals"])
    nc = build_program(cfg, plan)
    in_maps = prep_inputs(cfg, d["X"], d["W1"], d["b1"], d["W2"], d["b2"],
                          plan)
    return nc, in_maps


def run(cfg, X, W1, b1, W2, b2, vals, row, col, trace=False):
    nc, in_maps = make(cfg, dict(X=X, W1=W1, b1=b1, W2=W2, b2=b2,
                                 vals=vals, row=row, col=col))
    res = run_bass_kernel_spmd(nc, in_maps, list(range(cfg.M)), trace=trace)
    outs = [np.asarray(res.results[m]["out"]).T[:cfg.RPC]
            for m in range(cfg.M)]
    out = np.concatenate(outs, axis=0).astype(np.float32)
    return out, res


def kernel(X, W1, b1, W2, b2, vals, row, col):
    out, _ = run(FULL, X, W1, b1, W2, b2, vals, row, col)
    return out
